# revision 7
# baseline (speedup 1.0000x reference)
"""Trainium2 Bass kernel for nn_AbsoluteAttention (XLNet-style attention with
softmax over the HEAD axis n, faithful to the source module).

Reference math (fp32):
    s[i,j,b,n]  = (sum_d q[i,b,n,d] k[j,b,n,d]) * 0.125
    s          -= 1e30 * mask[i,j,b,1]          (broadcast over n!)
    p           = softmax over n (axis -1)
    out[i,b,n,d]= sum_j p[i,j,b,n] v[j,b,n,d]

Because the 1e30*mask shift is constant along the softmax axis n, masked
(i,j,b) entries become exactly -1e30 for ALL n (score absorbed by fp32
rounding) -> softmax yields exactly uniform 1/16.  Unmasked entries get a
plain softmax-over-n of the raw scaled scores.  So exactly:

    p = (1-m) * softmax_n(s) + m * (1/16)

Kernel strategy (8 cores = 2 batches x 4 i-chunks of 512):
    per core, per i-block of 256 (x2), per j-half of 1024 (x8 j-tiles of 128):
      scores   S^T[j,i] per n = K_n^T.T @ Q_n^T      (PE, fp32r, N=256)
      E        = exp(0.125 * S)                       (ACT, psum->sbuf, fp16)
      Z        = sum_n E   (pairwise tree)            (DVE, fp16 2x mode)
      W        = 1/Z                                  (DVE reciprocal approx)
      C        = W * (1-m)^T                          (DVE)
      P_n      = E_n * C       (in place)             (DVE)
      out_n^T += V_n.T-as-lhsT @ P_n  +  V_n lhsT @ (m/16)^T   (PE, fp16)
    merge j-halves in SBUF, DMA out^T -> out[i,n,d].

All input layout transforms (transposes / (1-m) / m/16 / fp16 casts) are done
host-side in numpy so every device DMA is a linear copy.
"""

import sys

import numpy as np

if "/opt/trn_rl_repo" not in sys.path:
    sys.path.insert(0, "/opt/trn_rl_repo")

SEQ, B, N, D = 2048, 2, 16, 64
SCALE = 0.125
NCORES = 8
IC = 512          # i-chunk per core (SEQ / 4 i-chunks)
IB = 256          # i columns per inner block
NIB = IC // IB    # 2
JT = 128          # j per tile (partition dim)
JH = 8            # j-tiles per j-half
NJH = (SEQ // JT) // JH  # 2
NG = 4            # heads per score-psum group

_CACHE = {}


def _build_nc():
    import concourse.bacc as bacc
    import concourse.mybir as mybir
    import concourse.tile as tile

    dt = mybir.dt
    f32, f16, f32r = dt.float32, dt.float16, dt.float32r
    Alu = mybir.AluOpType

    nc = bacc.Bacc("TRN2", target_bir_lowering=False, debug=False)

    # Per-core inputs (host pre-laid-out so every DMA is linear):
    #  qT : [64 d, 16 n, 512 i]            fp32   Q^T of this core's i-chunk
    #  kT : [16 jg, 64 d, 16 n, 128 j]     fp32   K^T tiles
    #  vp : [128 p, 16 n, 16 jg, 64 d]     fp16   V with j=jg*128+p
    #  at : [2 jh, 2 ib, 128 p, 8 jt, 256 i] fp16 (1-m)^T tiles
    #  mt : same layout                    fp16   (m/16)^T tiles
    qT = nc.dram_tensor("qT", [D, N, IC], f32r, kind="ExternalInput").ap()
    kT = nc.dram_tensor("kT", [SEQ // JT, D, N, JT], f32r, kind="ExternalInput").ap()
    vp = nc.dram_tensor("vp", [JT, N, SEQ // JT, D], f16, kind="ExternalInput").ap()
    at = nc.dram_tensor("at", [NJH, NIB, JT, JH, IB], f16, kind="ExternalInput").ap()
    mt = nc.dram_tensor("mt", [NJH, NIB, JT, JH, IB], f16, kind="ExternalInput").ap()
    # out^T layout [ib, d, n, i]; host transposes back to [i, n, d].
    out = nc.dram_tensor("out", [NIB, D, N, IB], f32, kind="ExternalOutput").ap()

    with tile.TileContext(nc) as tc:
        with (
            tc.tile_pool(name="consts", bufs=1) as consts,
            tc.tile_pool(name="kstream", bufs=2) as kpool,
            tc.tile_pool(name="epool", bufs=1) as epool,
            tc.tile_pool(name="zpool", bufs=1) as zpool,
            tc.tile_pool(name="maskp", bufs=2) as maskp,
            tc.tile_pool(name="small", bufs=2) as small,
            tc.tile_pool(name="opool", bufs=1) as opool,
            tc.tile_pool(name="spsum", bufs=3, space="PSUM") as spsum,
            tc.tile_pool(name="apsum", bufs=2, space="PSUM") as apsum,
        ):
            q_sb = consts.tile([D, N, IC], f32r)
            nc.sync.dma_start(q_sb[:], qT)
            v_sb = consts.tile([JT, N, SEQ // JT, D], f16)
            nc.sync.dma_start(v_sb[:], vp)

            for ib in range(NIB):
                o_sb = opool.tile([D, N, IB], f32, tag="o")
                for jh in range(NJH):
                    a_sb = maskp.tile([JT, JH, IB], f16, tag="a")
                    nc.sync.dma_start(a_sb[:], at[jh, ib])
                    m_sb = maskp.tile([JT, JH, IB], f16, tag="m")
                    nc.sync.dma_start(m_sb[:], mt[jh, ib])

                    E = epool.tile([JT, JH, N, IB], f16, tag="E")

                    for jt in range(JH):
                        jg = jh * JH + jt
                        k_sb = kpool.tile([D, N, JT], f32r, tag="k")
                        nc.sync.dma_start(k_sb[:], kT[jg])
                        for ng in range(N // NG):
                            ps = spsum.tile([JT, NG, IB], f32, tag="s")
                            for nn in range(NG):
                                n_ = ng * NG + nn
                                nc.tensor.matmul(
                                    ps[:, nn, :],
                                    k_sb[:, n_, :],
                                    q_sb[:, n_, ib * IB:(ib + 1) * IB],
                                    start=True,
                                    stop=True,
                                )
                            nc.scalar.activation(
                                E[:, jt, ng * NG:(ng + 1) * NG, :],
                                ps[:],
                                mybir.ActivationFunctionType.Exp,
                                scale=SCALE,
                            )

                    # Z = sum over n of E, pairwise tree, fp16 2x-mode ops.
                    # Done in two jt-half passes to keep the scratch small.
                    zf = zpool.tile([JT, JH, IB], f32, tag="zf")
                    cc = zpool.tile([JT, JH, IB], f16, tag="cc")
                    with nc.allow_low_precision("softmax partial sums in fp16"):
                        for jp in range(2):
                            js = slice(jp * 4, (jp + 1) * 4)
                            zb = zpool.tile([JT, 4, 8, IB], f16, tag="zb")
                            ev = E[:, js].rearrange(
                                "p j (h two) i -> p j two h i", two=2
                            )
                            nc.vector.tensor_tensor(
                                zb[:], ev[:, :, 0], ev[:, :, 1], Alu.add
                            )
                            zv = zb.rearrange("p j (h two) i -> p j two h i", two=2)
                            nc.vector.tensor_tensor(
                                zb[:, :, 0:4], zv[:, :, 0], zv[:, :, 1], Alu.add
                            )
                            zv2 = zb[:, :, 0:4].rearrange(
                                "p j (h two) i -> p j two h i", two=2
                            )
                            nc.vector.tensor_tensor(
                                zb[:, :, 0:2], zv2[:, :, 0], zv2[:, :, 1], Alu.add
                            )
                            nc.vector.tensor_tensor(
                                zf[:, js], zb[:, :, 0], zb[:, :, 1], Alu.add
                            )
                    nc.vector.reciprocal_approx_fast(out=zf[:], in_=zf[:])
                    nc.vector.tensor_tensor(cc[:], zf[:], a_sb[:], Alu.mult)

                    # normalize+mask in place: P_n = E_n * C
                    for n_ in range(N):
                        nc.vector.tensor_tensor(
                            E[:, :, n_, :], E[:, :, n_, :], cc[:], Alu.mult
                        )

                    # attention-value matmuls + uniform-mask term
                    for n_ in range(N):
                        acc = apsum.tile([D, IB], f32, tag="acc")
                        for jt in range(JH):
                            jg = jh * JH + jt
                            nc.tensor.matmul(
                                acc[:],
                                v_sb[:, n_, jg, :],
                                E[:, jt, n_, :],
                                start=(jt == 0),
                                stop=False,
                            )
                            nc.tensor.matmul(
                                acc[:],
                                v_sb[:, n_, jg, :],
                                m_sb[:, jt, :],
                                start=False,
                                stop=(jt == JH - 1),
                            )
                        if jh == 0:
                            nc.vector.tensor_copy(o_sb[:, n_, :], acc[:])
                        else:
                            nc.vector.tensor_tensor(
                                o_sb[:, n_, :], o_sb[:, n_, :], acc[:], Alu.add
                            )

                nc.sync.dma_start(out[ib], o_sb[:])

    nc.compile()
    return nc


def _prep_core_inputs(q_head, k_head, v_head, attn_mask):
    """Host-side shard + layout. Returns list of 8 per-core input dicts."""
    in_maps = []
    for c in range(NCORES):
        b = c // 4
        i0 = (c % 4) * IC
        q = q_head[i0:i0 + IC, b]                      # [512, 16, 64] fp32
        k = k_head[:, b]                               # [2048, 16, 64]
        v = v_head[:, b]                               # [2048, 16, 64]
        m = attn_mask[i0:i0 + IC, :, b, 0]             # [512 i, 2048 j]

        qTc = np.ascontiguousarray(q.transpose(2, 1, 0))        # [64, 16, 512]
        kTc = np.ascontiguousarray(
            k.reshape(SEQ // JT, JT, N, D).transpose(0, 3, 2, 1)
        )                                                       # [16, 64, 16, 128]
        vpc = np.ascontiguousarray(
            v.reshape(SEQ // JT, JT, N, D).transpose(1, 2, 0, 3)
        ).astype(np.float16)                                    # [128, 16, 16, 64]
        A = np.ascontiguousarray((1.0 - m).T)                   # [2048 j, 512 i]
        M = np.ascontiguousarray(m.T) * np.float32(1.0 / 16.0)
        atc = np.ascontiguousarray(
            A.reshape(NJH, JH, JT, NIB, IB).transpose(0, 3, 2, 1, 4)
        ).astype(np.float16)                                    # [2, 2, 128, 8, 256]
        mtc = np.ascontiguousarray(
            M.reshape(NJH, JH, JT, NIB, IB).transpose(0, 3, 2, 1, 4)
        ).astype(np.float16)
        in_maps.append({"qT": qTc, "kT": kTc, "vp": vpc, "at": atc, "mt": mtc})
    return in_maps


def run_on_cores(q_head, k_head, v_head, attn_mask, trace=False, **kw):
    from concourse.bass_utils import run_bass_kernel_spmd

    if "nc" not in _CACHE:
        _CACHE["nc"] = _build_nc()
    nc = _CACHE["nc"]
    in_maps = _prep_core_inputs(q_head, k_head, v_head, attn_mask)
    res = run_bass_kernel_spmd(
        nc, in_maps, core_ids=list(range(NCORES)), trace=trace, **kw
    )
    outs = np.empty((SEQ, B, N, D), dtype=np.float32)
    for c in range(NCORES):
        b = c // 4
        i0 = (c % 4) * IC
        # device out is [NIB, D, N, IB]; back to [i, n, d]
        oc = res.results[c]["out"].transpose(0, 3, 2, 1).reshape(IC, N, D)
        outs[i0:i0 + IC, b] = oc
    return outs, res


def kernel(q_head, k_head, v_head, attn_mask):
    out, _ = run_on_cores(
        np.asarray(q_head, dtype=np.float32),
        np.asarray(k_head, dtype=np.float32),
        np.asarray(v_head, dtype=np.float32),
        np.asarray(attn_mask, dtype=np.float32),
    )
    return out


# revision 27
# speedup vs baseline: 19983.7932x; 19983.7932x over previous
"""Trainium2 Bass kernel for nn_AbsoluteAttention (XLNet-style attention with
softmax over the HEAD axis n, faithful to the source module).

Reference math (fp32):
    s[i,j,b,n]  = (sum_d q[i,b,n,d] k[j,b,n,d]) * 0.125
    s          -= 1e30 * mask[i,j,b,1]          (broadcast over n!)
    p           = softmax over n (axis -1)
    out[i,b,n,d]= sum_j p[i,j,b,n] v[j,b,n,d]

Because the 1e30*mask shift is constant along the softmax axis n, masked
(i,j,b) entries become exactly -1e30 for ALL n (score absorbed by fp32
rounding) -> softmax yields exactly uniform 1/16.  Unmasked entries get a
plain softmax-over-n of the raw scaled scores.  So exactly:

    p = (1-m) * softmax_n(s) + m * (1/16)

Kernel strategy (8 cores = 2 batches x 4 i-chunks of 512):
    per core, per i-block of 256 (x2), per j-group of 512 (4 j-tiles of 128):
      scores   S^T[j,i] per n = K_n^T.T @ Q_n^T      (PE, fp32r, N=256)
      E        = exp(0.125 * S)                       (ACT, psum->sbuf, fp16)
      Z        = sum_n E   (pairwise tree)            (DVE, fp16 2x mode)
      W        = 1/Z                                  (DVE reciprocal approx)
      C        = W * (1-m)^T                          (DVE)
      P_n      = E_n * C       (in place)             (DVE)
      out_n^T += V_n-as-lhsT @ P_n  +  V_n @ (m/16)^T (PE, fp16)
    accumulate j-groups into SBUF out^T, DMA out^T; host transposes back.

E is double-buffered at j-group granularity so scores/exp of group g+1
overlap softmax (DVE) of group g and AV matmuls of group g-1.

All input layout transforms (transposes / (1-m) / m/16 / fp16 casts) are done
host-side in numpy so every device DMA is a linear copy.
"""

import sys

import numpy as np

if "/opt/trn_rl_repo" not in sys.path:
    sys.path.insert(0, "/opt/trn_rl_repo")

SEQ, B, N, D = 2048, 2, 16, 64
SCALE = 0.125
NCORES = 8
IC = 512          # i-chunk per core (SEQ / 4 i-chunks)
IB = 256          # i columns per inner block
NIB = IC // IB    # 2
JT = 128          # j per tile (partition dim)
JG = 4            # j-tiles per j-group
NJG = SEQ // (JT * JG)   # 4 groups
NG = 4            # heads per score-psum group

_CACHE = {}

import os
KREPS = int(os.environ.get("KREPS", "1"))  # repeat pipeline for differential timing


def _build_nc():
    import concourse.bacc as bacc
    import concourse.mybir as mybir
    import concourse.tile as tile

    dt = mybir.dt
    f32, f16, f32r = dt.float32, dt.float16, dt.float32r
    Alu = mybir.AluOpType

    nc = bacc.Bacc("TRN2", target_bir_lowering=False, debug=False)

    # Per-core inputs (host pre-laid-out so every DMA is linear):
    #  qT : [64 d, 16 n, 512 i]             f32r  Q^T of this core's i-chunk
    #  kT : [16 jg, 64 d, 16 n, 128 j]      f32r  K^T tiles
    #  vp : [128 p, 16 n, 16 jg, 64 d]      fp16  V with j=jg*128+p
    #  at : [4 g, 2 ib, 128 p, 4 jt, 256 i] fp16  (1-m)^T tiles
    #  mt : same layout                     fp16  (m/16)^T tiles
    qT = nc.dram_tensor("qT", [D, N, IC], f32r, kind="ExternalInput").ap()
    kT = nc.dram_tensor("kT", [SEQ // JT, D, N, JT], f32r, kind="ExternalInput").ap()
    vp = nc.dram_tensor("vp", [JT, N, SEQ // JT, D], f16, kind="ExternalInput").ap()
    at = nc.dram_tensor("at", [SEQ // JT, NIB, JT, IB], f16, kind="ExternalInput").ap()
    mt = nc.dram_tensor("mt", [SEQ // JT, NIB, JT, IB], f16, kind="ExternalInput").ap()
    # out^T layout [ib, d, n, i]; host transposes back to [i, n, d].
    out = nc.dram_tensor("out", [NIB, D, N, IB], f32, kind="ExternalOutput").ap()

    with tile.TileContext(nc) as tc:
        with (
            tc.tile_pool(name="consts", bufs=1) as consts,
            tc.tile_pool(name="kstream", bufs=3) as kpool,
            tc.tile_pool(name="epool", bufs=2) as epool,
            tc.tile_pool(name="zpool", bufs=2) as zpool,
            tc.tile_pool(name="maskp", bufs=3) as maskp,
            tc.tile_pool(name="opool", bufs=1) as opool,
            tc.tile_pool(name="spsum", bufs=3, space="PSUM") as spsum,
            tc.tile_pool(name="apsum", bufs=2, space="PSUM") as apsum,
        ):
            q_sb = consts.tile([D, N, IC], f32r)
            nc.sync.dma_start(q_sb[:, :, 0:IB], qT[:, :, 0:IB])
            v_sb = consts.tile([JT, N, SEQ // JT, D], f16)

            o_sbs = {}

            def scores_jt(ib, jg, jt, E):
                """PE scores + ACT exp for one j-tile (16 heads)."""
                k_sb = kpool.tile([D, N, JT], f32r, tag="k", name=f"k_{jg}_{ib}")
                nc.sync.dma_start(k_sb[:], kT[jg])
                for ng in range(N // NG):
                    ps = spsum.tile([JT, NG, IB], f32, tag="s", name=f"ps_{jg}_{ib}_{ng}")
                    for nn in range(NG):
                        n_ = ng * NG + nn
                        nc.tensor.matmul(
                            ps[:, nn, :],
                            k_sb[:, n_, :],
                            q_sb[:, n_, ib * IB:(ib + 1) * IB],
                            start=True,
                            stop=True,
                        )
                    nc.scalar.activation(
                        E[:, jt, ng * NG:(ng + 1) * NG, :],
                        ps[:],
                        mybir.ActivationFunctionType.Exp,
                        scale=SCALE,
                    )

            def softmax_block(E, a_sb, sz):
                """DVE: Z tree, reciprocal, C, in-place normalize of E."""
                zb_t = zpool.tile([JT, JG, 8, IB], f16, tag="zb")
                zb = zb_t[:, :sz]
                zf_t = zpool.tile([JT, JG, IB], f32, tag="zf")
                zf = zf_t[:, :sz]
                cc_t = zpool.tile([JT, JG, IB], f16, tag="cc")
                cc = cc_t[:, :sz]
                Ev = E[:, :sz]
                with nc.allow_low_precision("softmax partial sums in fp16"):
                    ev = Ev.rearrange("p j (h two) i -> p j two h i", two=2)
                    if sz > 2:
                        for jp in range(2):
                            js = slice(jp * (sz // 2), (jp + 1) * (sz // 2))
                            nc.vector.tensor_tensor(
                                zb[:, js], ev[:, js, 0], ev[:, js, 1], Alu.add
                            )
                    else:
                        nc.vector.tensor_tensor(
                            zb[:], ev[:, :, 0], ev[:, :, 1], Alu.add
                        )
                    zv = zb.rearrange("p j (h two) i -> p j two h i", two=2)
                    nc.vector.tensor_tensor(
                        zb[:, :, 0:4], zv[:, :, 0], zv[:, :, 1], Alu.add
                    )
                    zv2 = zb[:, :, 0:4].rearrange(
                        "p j (h two) i -> p j two h i", two=2
                    )
                    nc.vector.tensor_tensor(
                        zb[:, :, 0:2], zv2[:, :, 0], zv2[:, :, 1], Alu.add
                    )
                    nc.vector.tensor_tensor(zf[:], zb[:, :, 0], zb[:, :, 1], Alu.add)
                nc.vector.reciprocal_approx_fast(out=zf[:], in_=zf[:])
                nc.vector.tensor_tensor(cc[:], zf[:], a_sb[:, :sz], Alu.mult)
                for n4 in range(N // 4):
                    nc.vector.tensor_tensor(
                        Ev[:, :, n4 * 4:(n4 + 1) * 4, :],
                        Ev[:, :, n4 * 4:(n4 + 1) * 4, :],
                        cc[:, :, None, :].to_broadcast((JT, sz, 4, IB)),
                        Alu.mult,
                    )

            def av_pairs(blk, pairs):
                """PE attention-value matmuls + uniform-mask term + merge for
                the given head pairs.

                Two heads share one PSUM bank ([64, 2, IB] = 2KB): the second
                head's start=True only clears has_written bits, not the first
                head's accumulated data."""
                rep, ib, jg0, sz, first, last, E, m_sb = blk
                o_sb = o_sbs[(rep, ib)]
                for np_ in pairs:
                    acc = apsum.tile([D, 2, IB], f32, tag="acc",
                                     name=f"acc_{rep}_{ib}_{jg0}_{np_}")
                    for nn in range(2):
                        n_ = np_ * 2 + nn
                        for jt in range(sz):
                            jg = jg0 + jt
                            nc.tensor.matmul(
                                acc[:, nn, :],
                                v_sb[:, n_, jg, :],
                                E[:, jt, n_, :],
                                start=(jt == 0),
                                stop=False,
                                skip_group_check=True,
                            )
                            nc.tensor.matmul(
                                acc[:, nn, :],
                                v_sb[:, n_, jg, :],
                                m_sb[:, jt, :],
                                start=False,
                                stop=(jt == sz - 1),
                                skip_group_check=True,
                            )
                    sl = slice(np_ * 2, np_ * 2 + 2)
                    if first:
                        nc.scalar.copy(o_sb[:, sl, :], acc[:])
                    else:
                        nc.vector.tensor_tensor(
                            o_sb[:, sl, :], o_sb[:, sl, :], acc[:], Alu.add
                        )
                if last and pairs[-1] == N // 2 - 1:
                    nc.gpsimd.dma_start(out[ib, :, 0:8], o_sb[:, 0:8])
                    nc.gpsimd.dma_start(out[ib, :, 8:16], o_sb[:, 8:16])

            def split_pairs(nslots):
                """Distribute the 8 head-pairs across nslots jt iterations."""
                base = (N // 2) // nslots
                rem = (N // 2) % nslots
                chunks, p = [], 0
                for s in range(nslots):
                    c = base + (1 if s < rem else 0)
                    chunks.append(list(range(p, p + c)))
                    p += c
                return chunks

            # Software pipeline: the scores/exp of block g+1 are interleaved
            # jt-by-jt with the AV matmuls of block g, so the in-order PE
            # stream never stalls behind the DVE softmax chain.  First/last
            # blocks are small to shorten pipeline fill/drain.
            SIZES0 = [4, 4, 4, 4]
            SIZES1 = [4, 4, 4, 4]
            prev = None
            vload = 0
            for rep in range(KREPS):
              for ib in range(NIB):
                sizes = SIZES0 if ib == 0 else SIZES1
                o_sbs[(rep, ib)] = opool.tile(
                    [D, N, IB], f32, tag="o", name=f"o_sb_{rep}_{ib}"
                )
                jg0 = 0
                for bi, sz in enumerate(sizes):
                    a_sb = maskp.tile([JT, JG, IB], f16, tag="a",
                                      name=f"a_{rep}_{ib}_{bi}")
                    nc.sync.dma_start(a_sb[:, :sz], at[jg0:jg0 + sz, ib].rearrange("j p i -> p j i"))
                    m_sb = maskp.tile([JT, JG, IB], f16, tag="m",
                                      name=f"m_{rep}_{ib}_{bi}")
                    nc.sync.dma_start(m_sb[:, :sz], mt[jg0:jg0 + sz, ib].rearrange("j p i -> p j i"))
                    E = epool.tile([JT, JG, N, IB], f16, tag="E",
                                   name=f"E_{rep}_{ib}_{bi}")
                    chunks = split_pairs(sz)
                    for jt in range(sz):
                        scores_jt(ib, jg0 + jt, jt, E)
                        if rep == 0 and vload < 4 and ib == 0:
                            nc.gpsimd.dma_start(
                                v_sb[:, :, vload * 4:(vload + 1) * 4, :],
                                vp[:, :, vload * 4:(vload + 1) * 4, :],
                            )
                            vload += 1
                            if vload == 4:
                                nc.gpsimd.dma_start(
                                    q_sb[:, :, IB:IC], qT[:, :, IB:IC]
                                )
                        if prev is not None:
                            av_pairs(prev, chunks[jt])
                    softmax_block(E, a_sb, sz)
                    prev = (rep, ib, jg0, sz, bi == 0, bi == len(sizes) - 1,
                            E, m_sb)
                    jg0 += sz
            for ch in split_pairs(4):
                av_pairs(prev, ch)

    nc.compile()
    return nc


def _prep_core_inputs(q_head, k_head, v_head, attn_mask):
    """Host-side shard + layout. Returns list of 8 per-core input dicts."""
    in_maps = []
    for c in range(NCORES):
        b = c // 4
        i0 = (c % 4) * IC
        q = q_head[i0:i0 + IC, b]                      # [512, 16, 64] fp32
        k = k_head[:, b]                               # [2048, 16, 64]
        v = v_head[:, b]                               # [2048, 16, 64]
        m = attn_mask[i0:i0 + IC, :, b, 0]             # [512 i, 2048 j]

        qTc = np.ascontiguousarray(q.transpose(2, 1, 0))        # [64, 16, 512]
        kTc = np.ascontiguousarray(
            k.reshape(SEQ // JT, JT, N, D).transpose(0, 3, 2, 1)
        )                                                       # [16, 64, 16, 128]
        vpc = np.ascontiguousarray(
            v.reshape(SEQ // JT, JT, N, D).transpose(1, 2, 0, 3)
        ).astype(np.float16)                                    # [128, 16, 16, 64]
        A = np.ascontiguousarray((1.0 - m).T)                   # [2048 j, 512 i]
        M = np.ascontiguousarray(m.T) * np.float32(1.0 / 16.0)
        atc = np.ascontiguousarray(
            A.reshape(SEQ // JT, JT, NIB, IB).transpose(0, 2, 1, 3)
        ).astype(np.float16)                                    # [16, 2, 128, 256]
        mtc = np.ascontiguousarray(
            M.reshape(SEQ // JT, JT, NIB, IB).transpose(0, 2, 1, 3)
        ).astype(np.float16)
        in_maps.append({"qT": qTc, "kT": kTc, "vp": vpc, "at": atc, "mt": mtc})
    return in_maps


def run_on_cores(q_head, k_head, v_head, attn_mask, trace=False, **kw):
    from concourse.bass_utils import run_bass_kernel_spmd

    if "nc" not in _CACHE:
        _CACHE["nc"] = _build_nc()
    nc = _CACHE["nc"]
    in_maps = _prep_core_inputs(q_head, k_head, v_head, attn_mask)
    res = run_bass_kernel_spmd(
        nc, in_maps, core_ids=list(range(NCORES)), trace=trace, **kw
    )
    outs = np.empty((SEQ, B, N, D), dtype=np.float32)
    for c in range(NCORES):
        b = c // 4
        i0 = (c % 4) * IC
        # device out is [NIB, D, N, IB]; back to [i, n, d]
        oc = res.results[c]["out"].transpose(0, 3, 2, 1).reshape(IC, N, D)
        outs[i0:i0 + IC, b] = oc
    return outs, res


def kernel(q_head, k_head, v_head, attn_mask):
    out, _ = run_on_cores(
        np.asarray(q_head, dtype=np.float32),
        np.asarray(k_head, dtype=np.float32),
        np.asarray(v_head, dtype=np.float32),
        np.asarray(attn_mask, dtype=np.float32),
    )
    return out


# revision 47
# speedup vs baseline: 22071.1746x; 1.1045x over previous
"""Trainium2 Bass kernel for nn_AbsoluteAttention (XLNet-style attention with
softmax over the HEAD axis n, faithful to the source module).

Reference math (fp32):
    s[i,j,b,n]  = (sum_d q[i,b,n,d] k[j,b,n,d]) * 0.125
    s          -= 1e30 * mask[i,j,b,1]          (broadcast over n!)
    p           = softmax over n (axis -1)
    out[i,b,n,d]= sum_j p[i,j,b,n] v[j,b,n,d]

Because the 1e30*mask shift is constant along the softmax axis n, masked
(i,j,b) entries become exactly -1e30 for ALL n (score absorbed by fp32
rounding) -> softmax yields exactly uniform 1/16.  Unmasked entries get a
plain softmax-over-n of the raw scaled scores.  So exactly:

    p = (1-m) * softmax_n(s) + m * (1/16)

Kernel strategy (8 cores = 2 batches x 4 i-chunks of 512):
    per core, per i-block of 256 (x2), per j-group of 512 (4 j-tiles of 128):
      scores   S^T[j,i] per n = K_n^T.T @ Q_n^T      (PE, fp32r, N=256)
      E        = exp(0.125 * S)                       (ACT, psum->sbuf, fp16)
      Z        = sum_n E   (pairwise tree)            (DVE, fp16 2x mode)
      W        = 1/Z                                  (DVE reciprocal approx)
      C        = W * (1-m)^T                          (DVE)
      P_n      = E_n * C       (in place)             (DVE)
      out[i,n,:] += P_n-tile-as-lhsT @ V_n  +  (m/16)-tile @ V_n  (PE, fp16)
    transposed AV: out tiles are [128 i, 64 d] = 256B/partition per head, so
    all 16 heads' accumulators fit in 4 PSUM banks and stay resident across
    every j-group (no per-group merges; one ACT evacuation per i-block;
    natural [i,n,d] output).  Interleaved per-head accumulation in shared
    banks is safe: start=True only on each bank's first matmul; every other
    head's first touch relies on has_written=0 -> overwrite semantics.

E is triple-buffered at j-group granularity, and scores/exp of block g+1
are issued jt-by-jt interleaved with the AV matmuls of block g so the
in-order PE stream never stalls behind the DVE softmax chain.

All input layout transforms (transposes / (1-m) / m/16 / fp16 casts) are done
host-side in numpy so every device DMA is a linear copy.
"""

import sys

import numpy as np

if "/opt/trn_rl_repo" not in sys.path:
    sys.path.insert(0, "/opt/trn_rl_repo")

SEQ, B, N, D = 2048, 2, 16, 64
SCALE = 0.125
NCORES = 8
IC = 512          # i-chunk per core (SEQ / 4 i-chunks)
IB = 256          # i columns per inner block
NIB = IC // IB    # 2
JT = 128          # j per tile (partition dim)
JG = 4            # j-tiles per j-group
NJG = SEQ // (JT * JG)   # 4 groups
NG = 4            # heads per score-psum group

_CACHE = {}

import os
KREPS = int(os.environ.get("KREPS", "1"))  # repeat pipeline for differential timing


def _build_nc():
    import concourse.bacc as bacc
    import concourse.mybir as mybir
    import concourse.tile as tile

    dt = mybir.dt
    f32, f16, f32r = dt.float32, dt.float16, dt.float32r
    Alu = mybir.AluOpType

    nc = bacc.Bacc("TRN2", target_bir_lowering=False, debug=False)

    # Per-core inputs (host pre-laid-out so every DMA is linear):
    #  qT : [64 d, 16 n, 512 i]             f32r  Q^T of this core's i-chunk
    #  kT : [16 jg, 64 d, 16 n, 128 j]      f32r  K^T tiles
    #  vp : [128 p, 16 n, 16 jg, 64 d]      fp16  V with j=jg*128+p
    #  at : [16 jt, 2 ib, 128 p, 256 i]    fp16  (1-m)^T tiles
    #  mt : same layout                     fp16  (m/16)^T tiles
    qT = nc.dram_tensor("qT", [D, N, IC], f32r, kind="ExternalInput").ap()
    kT = nc.dram_tensor("kT", [SEQ // JT, D, N, JT], f32r, kind="ExternalInput").ap()
    vp = nc.dram_tensor("vp", [JT, N, SEQ // JT, D], f16, kind="ExternalInput").ap()
    at = nc.dram_tensor("at", [SEQ // JT, NIB, JT, IB], f16, kind="ExternalInput").ap()
    mt = nc.dram_tensor("mt", [SEQ // JT, NIB, JT, IB], f16, kind="ExternalInput").ap()
    # natural [i, n, d] output layout (transposed-AV writes i on partitions)
    out = nc.dram_tensor("out", [IC, N, D], f32, kind="ExternalOutput").ap()

    with tile.TileContext(nc) as tc:
        with (
            tc.tile_pool(name="consts", bufs=1) as consts,
            tc.tile_pool(name="kstream", bufs=2) as kpool,
            tc.tile_pool(name="epool", bufs=3) as epool,
            tc.tile_pool(name="zpool", bufs=2) as zpool,
            tc.tile_pool(name="maskp", bufs=2) as maskp,
            tc.tile_pool(name="opool", bufs=1) as opool,
            tc.tile_pool(name="spsum", bufs=2, space="PSUM") as spsum,
            tc.tile_pool(name="rpsum", bufs=1, space="PSUM") as rpsum,
        ):
            q_sb = consts.tile([D, N, IC], f32r)
            nc.sync.dma_start(q_sb[:, :, 0:IB], qT[:, :, 0:IB])
            v_sb = consts.tile([JT, N, SEQ // JT, D], f16)

            # PE warm-up: the HAM clock gate holds the PE at 1.2 GHz until
            # ~3.4us of sustained activity.  Burn dummy matmuls during the
            # initial DMA wait so the real scores start at full clock.
            wu = consts.tile([D, IB], f16, name="wu")
            nc.vector.memset(wu[:], 0.5)
            wups = spsum.tile([JT, NG, IB], f32, tag="s", name="wu_ps")
            for w in range(30):
                nc.tensor.matmul(
                    wups[:, 0, :], wu[:, 0:JT], wu[:],
                    start=True, stop=True, skip_group_check=True,
                )

            o_sbs = {}

            def scores_jt(ib, jg, jt, E):
                """PE scores + ACT exp for one j-tile (16 heads)."""
                k_sb = kpool.tile([D, N, JT], f32r, tag="k", name=f"k_{jg}_{ib}")
                nc.sync.dma_start(k_sb[:], kT[jg])
                for ng in range(N // NG):
                    ps = spsum.tile([JT, NG, IB], f32, tag="s", name=f"ps_{jg}_{ib}_{ng}")
                    for nn in range(NG):
                        n_ = ng * NG + nn
                        nc.tensor.matmul(
                            ps[:, nn, :],
                            k_sb[:, n_, :],
                            q_sb[:, n_, ib * IB:(ib + 1) * IB],
                            start=True,
                            stop=True,
                        )
                    nc.scalar.activation(
                        E[:, jt, ng * NG:(ng + 1) * NG, :],
                        ps[:],
                        mybir.ActivationFunctionType.Exp,
                        scale=SCALE,
                    )

            def softmax_block(E, a_sb, sz):
                """DVE: Z tree, reciprocal, C, in-place normalize of E."""
                zb_t = zpool.tile([JT, JG, 6, IB], f16, tag="zb")
                zb = zb_t[:, :sz]
                zf_t = zpool.tile([JT, JG, IB], f32, tag="zf")
                zf = zf_t[:, :sz]
                cc_t = zpool.tile([JT, JG, IB], f16, tag="cc")
                cc = cc_t[:, :sz]
                Ev = E[:, :sz]
                with nc.allow_low_precision("softmax partial sums in fp16"):
                    # heads 16 -> 8 -> 4 partial sums using 6 scratch slots:
                    # L1a: heads 0-7   -> zb[0:4]
                    # L2a: zb[0:4]     -> zb[0:2]
                    # L1b: heads 8-15  -> zb[2:6]
                    # L2b: zb[2:6]     -> zb[2:4]
                    # L3:  zb[0:2]+zb[2:4] -> zb[0:2]; L4 -> zf (fp32)
                    ev = Ev.rearrange("p j (h two) i -> p j two h i", two=2)
                    nc.vector.tensor_tensor(
                        zb[:, :, 0:4], ev[:, :, 0, 0:4], ev[:, :, 1, 0:4], Alu.add
                    )
                    za = zb[:, :, 0:4].rearrange("p j (h two) i -> p j two h i", two=2)
                    nc.vector.tensor_tensor(
                        zb[:, :, 0:2], za[:, :, 0], za[:, :, 1], Alu.add
                    )
                    nc.vector.tensor_tensor(
                        zb[:, :, 2:6], ev[:, :, 0, 4:8], ev[:, :, 1, 4:8], Alu.add
                    )
                    zc = zb[:, :, 2:6].rearrange("p j (h two) i -> p j two h i", two=2)
                    nc.vector.tensor_tensor(
                        zb[:, :, 2:4], zc[:, :, 0], zc[:, :, 1], Alu.add
                    )
                    nc.vector.tensor_tensor(
                        zb[:, :, 0:2], zb[:, :, 0:2], zb[:, :, 2:4], Alu.add
                    )
                    nc.vector.tensor_tensor(
                        zf[:], zb[:, :, 0], zb[:, :, 1], Alu.add
                    )
                nc.vector.reciprocal_approx_fast(out=zf[:], in_=zf[:])
                nc.vector.tensor_tensor(cc[:], zf[:], a_sb[:, :sz], Alu.mult)
                for n4 in range(N // 4):
                    nc.vector.tensor_tensor(
                        Ev[:, :, n4 * 4:(n4 + 1) * 4, :],
                        Ev[:, :, n4 * 4:(n4 + 1) * 4, :],
                        cc[:, :, None, :].to_broadcast((JT, sz, 4, IB)),
                        Alu.mult,
                    )

            def av_jt(blk, jt):
                """Transposed AV for one j-tile of the previous block: for
                each head, matmul(lhsT=P-tile [128j,128i], rhs=V [128j,64d])
                accumulating into the head's 256B slice of a resident PSUM
                accumulator [128i, 16n, 64d]; plus the (m/16)^T uniform-mask
                term.

                All 16 heads of one i-half share 2 banks and stay resident
                across every j-group of the i-block.  Only the very first
                matmul into each bank uses start=True (clears has_written
                for the whole bank); every other group's first-touch matmul
                relies on has_written=0 -> overwrite semantics.  PE executes
                matmuls strictly in program order, so this is safe."""
                rep, ib, jg0, sz, first, last, E, m_sb, accs = blk
                for ih in range(2):
                    acc = accs[ih]
                    for n_ in range(N):
                        nc.tensor.matmul(
                            acc[:, n_, :],
                            E[:, jt, n_, ih * 128:(ih + 1) * 128],
                            v_sb[:, n_, jg0 + jt, :],
                            start=(first and jt == 0 and n_ % 8 == 0),
                            stop=False,
                            skip_group_check=True,
                        )
                    for nh in range(2):
                        # one 512-col matmul covers 8 heads' acc regions
                        # (exactly one PSUM bank); rhs is V for heads
                        # nh*8..nh*8+7 of this j-tile, 2D-strided AP.
                        nc.tensor.matmul(
                            acc[:, nh * 8:(nh + 1) * 8, :],
                            m_sb[:, jt, ih * 128:(ih + 1) * 128],
                            v_sb[:, nh * 8:(nh + 1) * 8, jg0 + jt, :],
                            start=False,
                            stop=(last and jt == sz - 1),
                            skip_group_check=True,
                        )

            def evac(blk):
                """PSUM accumulators -> SBUF -> DRAM in natural [i,n,d]."""
                rep, ib, accs = blk[0], blk[1], blk[8]
                for ih in range(2):
                    o_sb = opool.tile([JT, N, D], f32, tag="o",
                                      name=f"o_{rep}_{ib}_{ih}")
                    nc.scalar.copy(o_sb[:], accs[ih][:])
                    i0 = ib * IB + ih * JT
                    nc.gpsimd.dma_start(out[i0:i0 + JT], o_sb[:])

            # Software pipeline: the scores/exp of block g+1 are interleaved
            # jt-by-jt with the AV matmuls of block g, so the in-order PE
            # stream never stalls behind the DVE softmax chain.
            prev = None
            vload = 0
            for rep in range(KREPS):
              for ib in range(NIB):
                last_ib = rep == KREPS - 1 and ib == NIB - 1
                SIZES = [4, 4, 4, 2, 2] if last_ib else [4, 4, 4, 4]
                acc0 = rpsum.tile([JT, N, D], f32, tag="acc0",
                                  name=f"acc_{rep}_{ib}_0")
                acc1 = rpsum.tile([JT, N, D], f32, tag="acc1",
                                  name=f"acc_{rep}_{ib}_1")
                accs = [acc0, acc1]
                jg0 = 0
                for bi, sz in enumerate(SIZES):
                    a_sb = maskp.tile([JT, JG, IB], f16, tag="a", bufs=1,
                                      name=f"a_{rep}_{ib}_{bi}")
                    nc.sync.dma_start(
                        a_sb[:, :sz],
                        at[jg0:jg0 + sz, ib].rearrange("j p i -> p j i"),
                    )
                    m_sb = maskp.tile([JT, JG, IB], f16, tag="m",
                                      name=f"m_{rep}_{ib}_{bi}")
                    nc.sync.dma_start(
                        m_sb[:, :sz],
                        mt[jg0:jg0 + sz, ib].rearrange("j p i -> p j i"),
                    )
                    E = epool.tile([JT, JG, N, IB], f16, tag="E",
                                   name=f"E_{rep}_{ib}_{bi}")
                    for jt in range(sz):
                        scores_jt(ib, jg0 + jt, jt, E)
                        if rep == 0 and vload < 4 and ib == 0:
                            nc.gpsimd.dma_start(
                                v_sb[:, :, vload * 4:(vload + 1) * 4, :],
                                vp[:, :, vload * 4:(vload + 1) * 4, :],
                            )
                            vload += 1
                            if vload == 4:
                                nc.gpsimd.dma_start(
                                    q_sb[:, :, IB:IC], qT[:, :, IB:IC]
                                )
                        if prev is not None:
                            psz = prev[3]
                            lo = jt * psz // sz
                            hi = (jt + 1) * psz // sz
                            for pj in range(lo, hi):
                                av_jt(prev, pj)
                            if prev[5] and hi == psz:
                                evac(prev)
                    softmax_block(E, a_sb, sz)
                    prev = (rep, ib, jg0, sz, bi == 0, bi == len(SIZES) - 1,
                            E, m_sb, accs)
                    jg0 += sz
            for jt in range(prev[3]):
                av_jt(prev, jt)
            evac(prev)

    nc.compile()
    return nc


def _prep_core_inputs(q_head, k_head, v_head, attn_mask):
    """Host-side shard + layout. Returns list of 8 per-core input dicts."""
    in_maps = []
    for c in range(NCORES):
        b = c // 4
        i0 = (c % 4) * IC
        q = q_head[i0:i0 + IC, b]                      # [512, 16, 64] fp32
        k = k_head[:, b]                               # [2048, 16, 64]
        v = v_head[:, b]                               # [2048, 16, 64]
        m = attn_mask[i0:i0 + IC, :, b, 0]             # [512 i, 2048 j]

        qTc = np.ascontiguousarray(q.transpose(2, 1, 0))        # [64, 16, 512]
        kTc = np.ascontiguousarray(
            k.reshape(SEQ // JT, JT, N, D).transpose(0, 3, 2, 1)
        )                                                       # [16, 64, 16, 128]
        vpc = np.ascontiguousarray(
            v.reshape(SEQ // JT, JT, N, D).transpose(1, 2, 0, 3)
        ).astype(np.float16)                                    # [128, 16, 16, 64]
        A = np.ascontiguousarray((1.0 - m).T)                   # [2048 j, 512 i]
        M = np.ascontiguousarray(m.T) * np.float32(1.0 / 16.0)
        atc = np.ascontiguousarray(
            A.reshape(SEQ // JT, JT, NIB, IB).transpose(0, 2, 1, 3)
        ).astype(np.float16)                                    # [16, 2, 128, 256]
        mtc = np.ascontiguousarray(
            M.reshape(SEQ // JT, JT, NIB, IB).transpose(0, 2, 1, 3)
        ).astype(np.float16)
        in_maps.append({"qT": qTc, "kT": kTc, "vp": vpc, "at": atc, "mt": mtc})
    return in_maps


def run_on_cores(q_head, k_head, v_head, attn_mask, trace=False, **kw):
    from concourse.bass_utils import run_bass_kernel_spmd

    if "nc" not in _CACHE:
        _CACHE["nc"] = _build_nc()
    nc = _CACHE["nc"]
    in_maps = _prep_core_inputs(q_head, k_head, v_head, attn_mask)
    res = run_bass_kernel_spmd(
        nc, in_maps, core_ids=list(range(NCORES)), trace=trace, **kw
    )
    outs = np.empty((SEQ, B, N, D), dtype=np.float32)
    for c in range(NCORES):
        b = c // 4
        i0 = (c % 4) * IC
        outs[i0:i0 + IC, b] = res.results[c]["out"]
    return outs, res


def kernel(q_head, k_head, v_head, attn_mask):
    out, _ = run_on_cores(
        np.asarray(q_head, dtype=np.float32),
        np.asarray(k_head, dtype=np.float32),
        np.asarray(v_head, dtype=np.float32),
        np.asarray(attn_mask, dtype=np.float32),
    )
    return out
